# revision 4
# baseline (speedup 1.0000x reference)
"""Trainium2 Bass kernel: causal GQA attention (prefill), 8-core tensor-parallel.

Problem: q [4096, 16*128], k/v [4096, 4*128], f32. 16 query heads, 4 kv heads,
head_dim 128, causal softmax(q k^T / sqrt(d)) v.

Sharding: head-parallel across 8 NeuronCores. Core c owns query heads
{2c, 2c+1}, which both belong to kv head c//2. Each core runs full causal
attention over its 2 heads; no cross-core communication.

Per-core kernel layout (N=4096 tokens, 32 tiles of 128):
  - qT, kT [d=128, 4096] bf16 in SBUF (PE-transposed on load).
  - vones [m=128, 32, 129] bf16: v tiles with a ones-column appended, so the
    PV matmul's output column 128 accumulates the softmax denominator.
  - Scores computed transposed: S^T[m, qcols] = kT_j.T @ qT  (PSUM f32),
    exp on ScalarE (scale=1/sqrt(d) folded in) -> pT [m, qcols] bf16, which is
    directly the stationary operand for PV: acc[q,129] += pT_j.T @ [v_j|1].
  - Causal: only blocks j<=i computed; diagonal blocks get an additive
    -1e9 mask before exp.
  - Normalize: out[q, d] = acc[:, :128] * reciprocal(acc[:, 128]).
"""

import sys

for _p in ("/opt/trn_rl_repo",):
    if _p not in sys.path:
        sys.path.insert(0, _p)

import numpy as np

import concourse.bacc as bacc
import concourse.bass as bass
import concourse.mybir as mybir
import concourse.tile as tile
from concourse.bass_utils import run_bass_kernel_spmd
from concourse.masks import make_identity

F32 = mybir.dt.float32
BF16 = mybir.dt.bfloat16

N = 4096
D = 128
H_PER_CORE = 2
NCORES = 8
NT = N // 128          # 32 token tiles
GQ = 4                 # q-tiles per group (512 query columns)
NG = NT // GQ          # 8 groups
SCALE = float(1.0 / np.sqrt(np.float32(D)))
MASK_VAL = -1e9


def _build():
    nc = bacc.Bacc(
        "TRN2",
        target_bir_lowering=False,
        debug=False,
        enable_asserts=False,
        num_devices=NCORES,
    )
    q_d = nc.dram_tensor("q", [N, H_PER_CORE * D], F32, kind="ExternalInput").ap()
    k_d = nc.dram_tensor("k", [N, D], F32, kind="ExternalInput").ap()
    v_d = nc.dram_tensor("v", [N, D], F32, kind="ExternalInput").ap()
    o_d = nc.dram_tensor("out", [N, H_PER_CORE * D], F32, kind="ExternalOutput").ap()

    with tile.TileContext(nc) as tc:
        with (
            tc.tile_pool(name="consts", bufs=1) as consts,
            tc.tile_pool(name="big", bufs=1) as big,
            tc.tile_pool(name="stage", bufs=4) as stage,
            tc.tile_pool(name="pstage", bufs=3) as pstage,
            tc.tile_pool(name="outp", bufs=4) as outp,
            tc.tile_pool(name="rpool", bufs=4) as rpool,
            tc.tile_pool(name="pst", bufs=3, space="PSUM") as psum_st,
            tc.tile_pool(name="pacc", bufs=1, space="PSUM") as psum_acc,
        ):
            identity = consts.tile([128, 128], F32)
            make_identity(nc, identity)

            # diag mask in S^T coords: mask[m, q] = 0 if m <= q else MASK_VAL
            maskd = consts.tile([128, 128], F32)
            nc.gpsimd.memset(maskd, 0.0)
            nc.gpsimd.affine_select(
                out=maskd,
                in_=maskd,
                compare_op=mybir.AluOpType.is_ge,
                fill=MASK_VAL,
                base=0,
                # keep 0 where (-1*m + 1*q) >= 0, i.e. m <= q
                pattern=[[1, 128]],
                channel_multiplier=-1,
            )

            qT = [big.tile([128, N], BF16, tag=f"qT{h}", name=f"qT{h}") for h in range(H_PER_CORE)]
            kT = big.tile([128, N], BF16, tag="kT")
            vones = big.tile([128, NT, 129], BF16, tag="vones")

            # ---- load + transpose q, k; load + cast v ----
            for h in range(H_PER_CORE):
                for i in range(NT):
                    s = stage.tile([128, 128], F32, tag="ld")
                    nc.sync.dma_start(
                        out=s, in_=q_d[i * 128 : (i + 1) * 128, h * D : (h + 1) * D]
                    )
                    tp = psum_st.tile([128, 128], F32, tag="st")
                    nc.tensor.transpose(tp, s, identity)
                    nc.vector.tensor_copy(qT[h][:, i * 128 : (i + 1) * 128], tp)
            for j in range(NT):
                s = stage.tile([128, 128], F32, tag="ld")
                nc.sync.dma_start(out=s, in_=k_d[j * 128 : (j + 1) * 128, :])
                tp = psum_st.tile([128, 128], F32, tag="st")
                nc.tensor.transpose(tp, s, identity)
                nc.vector.tensor_copy(kT[:, j * 128 : (j + 1) * 128], tp)
            for j in range(NT):
                s = stage.tile([128, 128], F32, tag="ld")
                nc.sync.dma_start(out=s, in_=v_d[j * 128 : (j + 1) * 128, :])
                nc.vector.tensor_copy(vones[:, j, 0:128], s)
                nc.vector.memset(vones[:, j, 128:129], 1.0)

            # ---- main attention loops ----
            for h in range(H_PER_CORE):
                for g in range(NG):
                    qc0 = g * GQ * 128  # first query column of this group
                    # one PSUM accumulator per q-tile; each in its own bank
                    # (matmul start=True clears has_written for the WHOLE bank,
                    # so accumulators must not share banks)
                    accs = [
                        psum_acc.tile([128, 129], F32, tag=f"acc{a}", name=f"acc{a}")
                        for a in range(GQ)
                    ]

                    for j in range(g * GQ + GQ):
                        kk = j - g * GQ  # >=0 when j is inside this group
                        if kk < 0:
                            c0, w, diag = qc0, GQ * 128, False
                        else:
                            c0, w, diag = qc0 + kk * 128, (GQ - kk) * 128, True
                        st = psum_st.tile([128, 512], F32, tag="st")
                        nc.tensor.matmul(
                            st[:, 0:w],
                            lhsT=kT[:, j * 128 : (j + 1) * 128],
                            rhs=qT[h][:, c0 : c0 + w],
                            start=True,
                            stop=True,
                        )
                        if diag:
                            nc.vector.tensor_add(st[:, 0:128], st[:, 0:128], maskd)
                        pt = pstage.tile([128, 512], BF16, tag="pt")
                        nc.scalar.activation(
                            out=pt[:, 0:w],
                            in_=st[:, 0:w],
                            func=mybir.ActivationFunctionType.Exp,
                            scale=SCALE,
                        )
                        for il in range(GQ):
                            i = g * GQ + il
                            if i < j:
                                continue
                            off = i * 128 - c0
                            nc.tensor.matmul(
                                accs[il][:, :],
                                lhsT=pt[:, off : off + 128],
                                rhs=vones[:, j, :],
                                start=(j == 0),
                                stop=(j == i),
                            )

                    for il in range(GQ):
                        i = g * GQ + il
                        acc = accs[il][:, :]
                        rec = rpool.tile([128, 1], F32, tag="rec")
                        nc.vector.reciprocal(rec, acc[:, 128:129])
                        ot = outp.tile([128, 128], F32, tag="ot")
                        nc.vector.tensor_scalar_mul(ot, acc[:, 0:128], rec)
                        nc.sync.dma_start(
                            out=o_d[i * 128 : (i + 1) * 128, h * D : (h + 1) * D],
                            in_=ot,
                        )

    nc.compile()
    return nc


_NC = None


def _get_nc():
    global _NC
    if _NC is None:
        _NC = _build()
    return _NC


def _shard(q, k, v):
    in_maps = []
    for c in range(NCORES):
        g = c // 2
        in_maps.append(
            {
                "q": np.ascontiguousarray(
                    q[:, c * H_PER_CORE * D : (c + 1) * H_PER_CORE * D],
                    dtype=np.float32,
                ),
                "k": np.ascontiguousarray(
                    k[:, g * D : (g + 1) * D], dtype=np.float32
                ),
                "v": np.ascontiguousarray(
                    v[:, g * D : (g + 1) * D], dtype=np.float32
                ),
            }
        )
    return in_maps


def _run(q, k, v, trace=False):
    nc = _get_nc()
    res = run_bass_kernel_spmd(
        nc, _shard(q, k, v), core_ids=list(range(NCORES)), trace=trace
    )
    out = np.concatenate(
        [np.asarray(res.results[c]["out"]) for c in range(NCORES)], axis=1
    )
    return out.astype(np.float32, copy=False), res


def kernel(q, k, v):
    out, _ = _run(np.asarray(q), np.asarray(k), np.asarray(v), trace=False)
    return out


# revision 5
# speedup vs baseline: 1.0258x; 1.0258x over previous
"""Trainium2 Bass kernel: causal GQA attention (prefill), 8-core tensor-parallel.

Problem: q [4096, 16*128], k/v [4096, 4*128], f32. 16 query heads, 4 kv heads,
head_dim 128, causal softmax(q k^T / sqrt(d)) v.

Sharding: head-parallel across 8 NeuronCores. Core c owns query heads
{2c, 2c+1}, which both belong to kv head c//2. Each core runs full causal
attention over its 2 heads; no cross-core communication.

Per-core kernel (N=4096 tokens, 32 tiles of 128):
  - Inputs DMA'd in large chunks into f32 staging, then PE-transposed into
    qT/kT [d=128, 4096] bf16. v cast to bf16 with a ones-column appended
    (vones), so the PV matmul's output column 128 accumulates the softmax
    denominator.
  - Scores computed transposed: S^T[m, qcols] = kT_j.T @ qT (PSUM f32), exp on
    ScalarE (scale=1/sqrt(d) folded in) over wide [128, <=1024] strips ->
    pT [m, qcols] bf16, which is directly the stationary operand for PV:
    acc[q,129] += pT_j.T @ [v_j|1].
  - Causal: only blocks j<=i computed; diagonal blocks get an additive
    -1e9 mask before exp. No max-subtraction (scores ~ N(0,1): exp is safe).
  - Normalize: out[q, d] = acc[:, :128] * reciprocal(acc[:, 128]).
  - Emission order is pipelined: transposes/casts for the next group are
    emitted between attention groups so TensorE never drains (HAM stays warm)
    and the main loop starts within a few microseconds.
"""

import sys

for _p in ("/opt/trn_rl_repo",):
    if _p not in sys.path:
        sys.path.insert(0, _p)

import numpy as np

import concourse.bacc as bacc
import concourse.mybir as mybir
import concourse.tile as tile
from concourse.bass_utils import run_bass_kernel_spmd
from concourse.masks import make_identity

F32 = mybir.dt.float32
BF16 = mybir.dt.bfloat16

N = 4096
D = 128
H_PER_CORE = 2
NCORES = 8
NT = N // 128          # 32 token tiles
GQ = 4                 # q-tiles per group (512 query columns)
NG = NT // GQ          # 8 groups
SCALE = float(1.0 / np.sqrt(np.float32(D)))
MASK_VAL = -1e9
DMA_CHUNK = 8          # token tiles per input DMA instruction


def _build():
    nc = bacc.Bacc(
        "TRN2",
        target_bir_lowering=False,
        debug=False,
        enable_asserts=False,
        num_devices=NCORES,
    )
    q_d = nc.dram_tensor("q", [N, H_PER_CORE * D], F32, kind="ExternalInput").ap()
    k_d = nc.dram_tensor("k", [N, D], F32, kind="ExternalInput").ap()
    v_d = nc.dram_tensor("v", [N, D], F32, kind="ExternalInput").ap()
    o_d = nc.dram_tensor("out", [N, H_PER_CORE * D], F32, kind="ExternalOutput").ap()

    with tile.TileContext(nc) as tc:
        with (
            tc.tile_pool(name="consts", bufs=1) as consts,
            tc.tile_pool(name="big", bufs=1) as big,
            tc.tile_pool(name="pstage", bufs=3) as pstage,
            tc.tile_pool(name="outp", bufs=4) as outp,
            tc.tile_pool(name="rpool", bufs=4) as rpool,
            tc.tile_pool(name="pst", bufs=2, space="PSUM") as psum_st,
            tc.tile_pool(name="pacc", bufs=1, space="PSUM") as psum_acc,
        ):
            identity = consts.tile([128, 128], F32)
            make_identity(nc, identity)

            # diag mask in S^T coords: mask[m, q] = 0 if m <= q else MASK_VAL
            maskd = consts.tile([128, 128], F32)
            nc.gpsimd.memset(maskd, 0.0)
            nc.gpsimd.affine_select(
                out=maskd,
                in_=maskd,
                compare_op=mybir.AluOpType.is_ge,
                fill=MASK_VAL,
                base=0,
                pattern=[[1, 128]],
                channel_multiplier=-1,
            )

            # f32 staging for transposes ([p, tile, col], p = token % 128)
            qst = big.tile([128, NT, H_PER_CORE * D], F32, tag="qst")
            kst = big.tile([128, NT, D], F32, tag="kst")
            vst = big.tile([128, NT, D], F32, tag="vst")

            qT = [
                big.tile([128, N], BF16, tag=f"qT{h}", name=f"qT{h}")
                for h in range(H_PER_CORE)
            ]
            kT = big.tile([128, N], BF16, tag="kT")
            vones = big.tile([128, NT, 129], BF16, tag="vones")

            # ---- chunked input DMAs (big descriptors, few instructions) ----
            for t0 in range(0, NT, DMA_CHUNK):
                r0, r1 = t0 * 128, (t0 + DMA_CHUNK) * 128
                nc.sync.dma_start(
                    out=qst[:, t0 : t0 + DMA_CHUNK, :],
                    in_=q_d[r0:r1, :].rearrange("(t p) c -> p t c", p=128),
                )
                nc.sync.dma_start(
                    out=kst[:, t0 : t0 + DMA_CHUNK, :],
                    in_=k_d[r0:r1, :].rearrange("(t p) c -> p t c", p=128),
                )
                nc.sync.dma_start(
                    out=vst[:, t0 : t0 + DMA_CHUNK, :],
                    in_=v_d[r0:r1, :].rearrange("(t p) c -> p t c", p=128),
                )

            def prep_tile(t, do_kv, qh_list):
                """Emit transpose/cast work for token tile t."""
                if do_kv:
                    tp = psum_st.tile([128, 128], F32, tag="st", name="tpk")
                    nc.tensor.transpose(tp, kst[:, t, :], identity)
                    nc.vector.tensor_copy(kT[:, t * 128 : (t + 1) * 128], tp)
                    nc.vector.tensor_copy(vones[:, t, 0:128], vst[:, t, :])
                    nc.vector.memset(vones[:, t, 128:129], 1.0)
                for h in qh_list:
                    tp = psum_st.tile([128, 128], F32, tag="st", name="tpq")
                    nc.tensor.transpose(tp, qst[:, t, h * D : (h + 1) * D], identity)
                    nc.vector.tensor_copy(qT[h][:, t * 128 : (t + 1) * 128], tp)

            # prologue: everything group (h=0, g=0) needs
            for t in range(GQ):
                prep_tile(t, True, [0])

            def attention_group(h, g):
                qc0 = g * GQ * 128
                # one PSUM accumulator per q-tile, each in its own bank
                # (matmul start=True clears has_written for the whole bank)
                accs = [
                    psum_acc.tile([128, 129], F32, tag=f"acc{a}", name=f"acc{a}")
                    for a in range(GQ)
                ]

                # strips: groups of score blocks evaluated with one wide exp.
                # Each block: (j, c0, w) with columns [c0, c0+w) of this head's
                # query range; in-group blocks (j >= 4g) get a diagonal mask.
                blocks = [(j, qc0, GQ * 128) for j in range(g * GQ)]
                blocks += [
                    (g * GQ + kk, qc0 + kk * 128, (GQ - kk) * 128)
                    for kk in range(GQ)
                ]
                strips = [blocks[x : x + 2] for x in range(0, len(blocks), 2)]

                for strip in strips:
                    st2 = psum_st.tile([128, 1024], F32, tag="st", name="st2")
                    pt2 = pstage.tile([128, 1024], BF16, tag="pt", name="pt2")
                    so = 0
                    offs = []
                    for j, c0, w in strip:
                        nc.tensor.matmul(
                            st2[:, so : so + w],
                            lhsT=kT[:, j * 128 : (j + 1) * 128],
                            rhs=qT[h][:, c0 : c0 + w],
                            start=True,
                            stop=True,
                        )
                        if j >= g * GQ:  # diagonal block: causal mask
                            nc.vector.tensor_add(
                                st2[:, so : so + 128], st2[:, so : so + 128], maskd
                            )
                        offs.append(so)
                        so += w
                    nc.scalar.activation(
                        out=pt2[:, 0:so],
                        in_=st2[:, 0:so],
                        func=mybir.ActivationFunctionType.Exp,
                        scale=SCALE,
                    )
                    for (j, c0, w), so_b in zip(strip, offs):
                        for il in range(GQ):
                            i = g * GQ + il
                            if i < j:
                                continue
                            off = so_b + i * 128 - c0
                            nc.tensor.matmul(
                                accs[il],
                                lhsT=pt2[:, off : off + 128],
                                rhs=vones[:, j, :],
                                start=(j == 0),
                                stop=(j == i),
                            )

                for il in range(GQ):
                    i = g * GQ + il
                    rec = rpool.tile([128, 1], F32, tag="rec", name="rec")
                    nc.vector.reciprocal(rec, accs[il][:, 128:129])
                    ot = outp.tile([128, 128], F32, tag="ot", name="ot")
                    nc.vector.tensor_scalar_mul(ot, accs[il][:, 0:128], rec)
                    nc.sync.dma_start(
                        out=o_d[i * 128 : (i + 1) * 128, h * D : (h + 1) * D],
                        in_=ot,
                    )

            # ---- main loops with rolling prologues ----
            for h in range(H_PER_CORE):
                for g in range(NG):
                    attention_group(h, g)
                    if h == 0:
                        if g < NG - 1:
                            # next group's k/v/q tiles
                            for t in range(GQ * (g + 1), GQ * (g + 2)):
                                prep_tile(t, True, [0])
                            # spread head-1 q transposes over groups 3..6
                            if 3 <= g <= 6:
                                for t in range(GQ * 2 * (g - 3), GQ * 2 * (g - 2)):
                                    prep_tile(t, False, [1])
                        else:
                            # last head-1 tiles
                            for t in range(GQ * 2 * 4, NT):
                                prep_tile(t, False, [1])

    nc.compile()
    return nc


_NC = None


def _get_nc():
    global _NC
    if _NC is None:
        _NC = _build()
    return _NC


def _shard(q, k, v):
    in_maps = []
    for c in range(NCORES):
        g = c // 2
        in_maps.append(
            {
                "q": np.ascontiguousarray(
                    q[:, c * H_PER_CORE * D : (c + 1) * H_PER_CORE * D],
                    dtype=np.float32,
                ),
                "k": np.ascontiguousarray(k[:, g * D : (g + 1) * D], dtype=np.float32),
                "v": np.ascontiguousarray(v[:, g * D : (g + 1) * D], dtype=np.float32),
            }
        )
    return in_maps


def _run(q, k, v, trace=False):
    nc = _get_nc()
    res = run_bass_kernel_spmd(
        nc, _shard(q, k, v), core_ids=list(range(NCORES)), trace=trace
    )
    out = np.concatenate(
        [np.asarray(res.results[c]["out"]) for c in range(NCORES)], axis=1
    )
    return out.astype(np.float32, copy=False), res


def kernel(q, k, v):
    out, _ = _run(np.asarray(q), np.asarray(k), np.asarray(v), trace=False)
    return out
